# revision 1
# baseline (speedup 1.0000x reference)
"""VQ codebook kernel for Trainium2 (8 NeuronCores, Bass/Tile).

Problem: features [131072, 128] f32, codes [2048, 128] f32.
Output: codes[argmin_k ||f - c_k||^2] -> [131072, 128] f32.

Strategy (data-parallel per sharding hint): shard features N across the 8
cores (16384 rows each), replicate the codebook. Per core:
  - preamble: transpose codes into codesT [d=128, k=2048] on the tensor
    engine; build minus_half_csq_rep [128, 2048] = -||c_k||^2/2 broadcast
    across partitions via two small matmul passes (ones-vector tricks).
  - per 128-row feature tile: PE transposes the tile, then 4 fp32 matmuls
    compute dot = f @ c^T into PSUM ([128, 2048], two [128,1024] halves
    double-buffered). argmin_k dist = argmax_k (dot - csq/2): VectorE
    tensor_tensor_reduce fuses the bias add + PSUM->SBUF copy + running
    max per half; max_index then finds the first index matching the row
    max (same tie-break as jnp.argmin). GPSIMD indirect DMA gathers
    codes[idx] rows straight from DRAM; DMA writes the output tile.

fp32 matmul is used throughout: measured on HW it is fp32-accurate
(rel err ~2e-7), which keeps argmin flips vs the fp32 reference at ~0.
"""

import os
import sys

import numpy as np

for _p in ("/opt/trn_rl_repo", "/root/.axon_site/_ro/trn_rl_repo"):
    if os.path.isdir(_p) and _p not in sys.path:
        sys.path.insert(0, _p)

import concourse.bacc as bacc
import concourse.bass as bass
import concourse.mybir as mybir
import concourse.tile as tile
from concourse.bass_utils import run_bass_kernel_spmd

N, K, D = 131072, 2048, 128
N_CORES = 8
N_SHARD = N // N_CORES          # 16384
M_TILES = N_SHARD // 128        # 128
K_CHUNK = 512                   # max fp32 moving free dim / one PSUM bank
NEG_INF = -3.0e38
POS_INF = 3.0e38

_compiled = None


def _build(n_shard=N_SHARD, num_devices=N_CORES, stage=4,
           variant="native3"):
    m_tiles = n_shard // 128
    nc = bacc.Bacc("TRN2", target_bir_lowering=False, debug=False,
                   num_devices=num_devices)
    f32 = mybir.dt.float32

    features = nc.dram_tensor("features", [n_shard, D], f32,
                              kind="ExternalInput").ap()
    codes = nc.dram_tensor("codes", [K, D], f32, kind="ExternalInput").ap()
    ident = nc.dram_tensor("identity", [128, 128], f32,
                           kind="ExternalInput").ap()
    out = nc.dram_tensor("out", [n_shard, D], f32,
                         kind="ExternalOutput").ap()
    idx_out = nc.dram_tensor("idx_out", [n_shard, 1], mybir.dt.uint32,
                             kind="ExternalOutput").ap()

    with tile.TileContext(nc) as tc:
        with (
            tc.tile_pool(name="const", bufs=1) as const_pool,
            tc.tile_pool(name="fin", bufs=3) as fin_pool,
            tc.tile_pool(name="ft", bufs=2) as ft_pool,
            tc.tile_pool(name="score", bufs=2) as score_pool,
            tc.tile_pool(name="small", bufs=3) as small_pool,
            tc.tile_pool(name="gath", bufs=3) as gath_pool,
            tc.tile_pool(name="pdot", bufs=2, space="PSUM") as pdot_pool,
            tc.tile_pool(name="ptr", bufs=2, space="PSUM") as ptr_pool,
        ):
            ident_sb = const_pool.tile([128, 128], f32)
            nc.sync.dma_start(ident_sb[:], ident[:])

            # --- codesT [d=128, k=2048] via 16 PE transposes ---
            codesT = const_pool.tile([128, K], f32)
            for t in range(K // 128):
                ct_in = fin_pool.tile([128, 128], f32, tag="ctin")
                nc.sync.dma_start(ct_in[:], codes[t * 128:(t + 1) * 128, :])
                ct_ps = ptr_pool.tile([128, 128], f32, tag="tr")
                nc.tensor.transpose(ct_ps[:], ct_in[:], ident_sb[:])
                nc.scalar.copy(codesT[:, t * 128:(t + 1) * 128], ct_ps[:])

            # --- csq_row [1, 2048] = sum_d codesT^2 via ones matmul ---
            sq = const_pool.tile([128, K], f32)
            nc.vector.tensor_tensor(out=sq[:], in0=codesT[:], in1=codesT[:],
                                    op=mybir.AluOpType.mult)
            ones_col = const_pool.tile([128, 1], f32)
            nc.vector.memset(ones_col[:], 1.0)
            ones_row = const_pool.tile([1, 128], f32)
            nc.vector.memset(ones_row[:], 1.0)
            csq_row = const_pool.tile([1, K], f32)
            for c in range(K // K_CHUNK):
                sl = slice(c * K_CHUNK, (c + 1) * K_CHUNK)
                csq_ps = ptr_pool.tile([1, K_CHUNK], f32, tag="tr")
                nc.tensor.matmul(csq_ps[:], ones_col[:], sq[:, sl],
                                 start=True, stop=True)
                # scale by -0.5 while evacuating PSUM
                nc.scalar.mul(csq_row[:, sl], csq_ps[:], -0.5)

            # --- broadcast -csq/2 across partitions: [128, 2048] ---
            nhcsq = const_pool.tile([128, K], f32)
            for c in range(K // K_CHUNK):
                sl = slice(c * K_CHUNK, (c + 1) * K_CHUNK)
                b_ps = ptr_pool.tile([128, K_CHUNK], f32, tag="tr")
                nc.tensor.matmul(b_ps[:], ones_row[:], csq_row[:, sl],
                                 start=True, stop=True)
                nc.scalar.copy(nhcsq[:, sl], b_ps[:])

            # --- iota_desc [128, 2048] f32: value at k is (K-1) - k ---
            iota_i = const_pool.tile([128, K], mybir.dt.int32)
            nc.gpsimd.iota(iota_i[:], pattern=[[-1, K]], base=K - 1,
                           channel_multiplier=0)
            iota_desc = const_pool.tile([128, K], f32)
            nc.vector.tensor_copy(iota_desc[:], iota_i[:])

            # --- main loop over feature tiles ---
            for i in range(m_tiles):
                rows = slice(i * 128, (i + 1) * 128)
                f_in = fin_pool.tile([128, 128], f32, tag="fin")
                nc.sync.dma_start(f_in[:], features[rows, :])
                fT_ps = ptr_pool.tile([128, 128], f32, tag="tr")
                nc.tensor.transpose(fT_ps[:], f_in[:], ident_sb[:])
                fT = ft_pool.tile([128, 128], f32)
                nc.scalar.copy(fT[:], fT_ps[:])

                # nscore = csq/2 - dot (distance up to a per-row constant;
                # argmin + first-index tie-break match jnp.argmin exactly)
                nscore = score_pool.tile([128, K], f32)
                hm = small_pool.tile([128, 2], f32, tag="hm")
                for h in range(2):
                    hsl = slice(h * 1024, (h + 1) * 1024)
                    dot_ps = pdot_pool.tile([128, 1024], f32, tag="dot")
                    for c in range(2):
                        ksl = slice(h * 1024 + c * K_CHUNK,
                                    h * 1024 + (c + 1) * K_CHUNK)
                        psl = slice(c * K_CHUNK, (c + 1) * K_CHUNK)
                        nc.tensor.matmul(dot_ps[:, psl], fT[:],
                                         codesT[:, ksl],
                                         start=True, stop=True)
                    if variant == "ttrmin2":
                        # fused: nscore = -(dot + nhcsq); half-min accum
                        nc.vector.tensor_tensor_reduce(
                            out=nscore[:, hsl],
                            in0=dot_ps[:],
                            in1=nhcsq[:, hsl],
                            scale=-1.0,
                            scalar=POS_INF,
                            op0=mybir.AluOpType.add,
                            op1=mybir.AluOpType.min,
                            accum_out=hm[:, h:h + 1],
                        )
                    else:
                        nc.vector.scalar_tensor_tensor(
                            out=nscore[:, hsl],
                            in0=dot_ps[:],
                            scalar=-1.0,
                            in1=nhcsq[:, hsl],
                            op0=mybir.AluOpType.mult,
                            op1=mybir.AluOpType.subtract,
                        )
                m_val = small_pool.tile([128, 1], f32, tag="m")
                if variant == "ttrmin2":
                    nc.vector.tensor_tensor(out=m_val[:], in0=hm[:, 0:1],
                                            in1=hm[:, 1:2],
                                            op=mybir.AluOpType.min)
                else:
                    nc.vector.tensor_reduce(out=m_val[:], in_=nscore[:],
                                            axis=mybir.AxisListType.X,
                                            op=mybir.AluOpType.min)
                # acc = sum((nscore <= m) * iota_desc) = (K-1) - idx
                junk = score_pool.tile([128, K], f32, tag="junk")
                acc = small_pool.tile([128, 1], f32, tag="acc")
                nc.vector.scalar_tensor_tensor(
                    out=junk[:],
                    in0=nscore[:],
                    scalar=m_val[:],
                    in1=iota_desc[:],
                    op0=mybir.AluOpType.is_le,
                    op1=mybir.AluOpType.mult,
                    accum_out=acc[:],
                )
                idx_f = small_pool.tile([128, 1], f32, tag="idxf")
                nc.vector.tensor_scalar(
                    out=idx_f[:], in0=acc[:], scalar1=float(K - 1),
                    scalar2=-1.0, op0=mybir.AluOpType.subtract,
                    op1=mybir.AluOpType.mult)
                idx_u = small_pool.tile([128, 1], mybir.dt.uint32, tag="idxu")
                nc.vector.tensor_copy(idx_u[:], idx_f[:])
                nc.sync.dma_start(idx_out[rows, :], idx_u[:])
                if stage < 4:
                    nc.sync.dma_start(out[rows, :], nscore[:, 0:D])
                    continue
                gath = gath_pool.tile([128, D], f32)
                nc.gpsimd.indirect_dma_start(
                    out=gath[:],
                    out_offset=None,
                    in_=codes[:],
                    in_offset=bass.IndirectOffsetOnAxis(ap=idx_u[:, 0:1],
                                                        axis=0),
                )
                nc.sync.dma_start(out[rows, :], gath[:])
    nc.compile()
    return nc


def _get_compiled():
    global _compiled
    if _compiled is None:
        _compiled = _build()
    return _compiled


def kernel(features: np.ndarray, codes: np.ndarray,
           _trace: bool = False, _results_box: list | None = None
           ) -> np.ndarray:
    features = np.ascontiguousarray(features, dtype=np.float32)
    codes = np.ascontiguousarray(codes, dtype=np.float32)
    assert features.shape == (N, D) and codes.shape == (K, D)

    nc = _get_compiled()
    ident = np.eye(128, dtype=np.float32)
    in_maps = [
        {
            "features": features[c * N_SHARD:(c + 1) * N_SHARD],
            "codes": codes,
            "identity": ident,
        }
        for c in range(N_CORES)
    ]
    res = run_bass_kernel_spmd(nc, in_maps, list(range(N_CORES)),
                               trace=_trace)
    if _results_box is not None:
        _results_box.append(res)
    out = np.concatenate([res.results[c]["out"] for c in range(N_CORES)],
                         axis=0)
    return out


if __name__ == "__main__":
    rng = np.random.default_rng(0)
    f = rng.standard_normal((N, D)).astype(np.float32)
    c = rng.standard_normal((K, D)).astype(np.float32)
    got = kernel(f, c)
    d = (f ** 2).sum(1)[:, None] - 2.0 * (f @ c.T) + (c ** 2).sum(1)
    want = c[np.argmin(d, axis=1)]
    err = np.abs(got - want)
    rel = np.linalg.norm(got - want) / np.linalg.norm(want)
    print(f"maxabs={err.max():.3e} rel={rel:.3e} "
          f"badrows={(err.max(1) > 1e-4).sum()}")



# revision 2
# speedup vs baseline: 1.8550x; 1.8550x over previous
"""VQ codebook kernel for Trainium2 (8 NeuronCores, Bass/Tile).

Problem: features [131072, 128] f32, codes [2048, 128] f32.
Output: codes[argmin_k ||f - c_k||^2] -> [131072, 128] f32.

Strategy (data-parallel): shard features N across the 8 cores (16384 rows
each), replicate the codebook. argmin_k dist = argmax_k score where
score = f.c_k - ||c_k||^2/2.

Host-side prep: features/codes/bias are split into fp32r (RNE to 11
mantissa bits) hi/lo pairs, and the feature shards are pre-transposed, so
the on-chip work per 128-row tile is:
  - PE: dot = f.c^T via compensated fp32r matmuls (hi*hi + hi*lo + lo*hi,
    1 cycle/row each vs 4 for fp32; error ~2^-22 keeps argmax flips ~0).
    For k-chunks 0-1 the -csq/2 bias is also accumulated into PSUM via
    rank-1 fp32r (hi+lo) matmuls with a ones stationary vector.
  - DVE: scalar_tensor_tensor adds the fp32 bias for chunks 2-3
    (PSUM->SBUF); then a running-max tensor_tensor_scan over chunks 0-1
    (straight from PSUM) chained into chunks 2-3 gives r[t] and the row
    max M = r[-1].
  - ACT: idx = sum_t sign(M - r[t]) via one Sign activation with
    accum_out (count of positions before the first max = argmax index,
    matching jnp.argmin first-index tie-break).
  - GPSIMD indirect DMA gathers codes[idx] rows from DRAM; DMA writes the
    output tile.

Engine busy/tile ~ DVE 3.5us, PE 3.4us, ACT 2.3us, Pool 1.2us.
"""

import os
import sys

import numpy as np

for _p in ("/opt/trn_rl_repo", "/root/.axon_site/_ro/trn_rl_repo"):
    if os.path.isdir(_p) and _p not in sys.path:
        sys.path.insert(0, _p)

import concourse.bacc as bacc
import concourse.bass as bass
import concourse.mybir as mybir
import concourse.tile as tile
from concourse.bass_utils import run_bass_kernel_spmd

N, K, D = 131072, 2048, 128
N_CORES = 8
N_SHARD = N // N_CORES          # 16384
M_TILES = N_SHARD // 128        # 128
KC = 512                        # matmul chunk (one PSUM bank group)
KH = 1024                       # bias split: chunks 0-1 PE-folded, 2-3 DVE
NEG_INF = -3.0e38

_compiled = None


def _rne11(x: np.ndarray) -> np.ndarray:
    """Round fp32 to fp32r: RNE to 11 mantissa bits (drop low 12)."""
    b = np.ascontiguousarray(x, dtype=np.float32).view(np.uint32)
    keep = np.uint32(0xFFFFF000)
    half = np.uint32(0x800)
    tie = (b >> np.uint32(12)) & np.uint32(1)
    r = (b + half - np.uint32(1) + tie) & keep
    return r.view(np.float32)


def _build(n_shard=N_SHARD, num_devices=N_CORES):
    m_tiles = n_shard // 128
    nc = bacc.Bacc("TRN2", target_bir_lowering=False, debug=False,
                   num_devices=num_devices)
    f32 = mybir.dt.float32
    f32r = mybir.dt.float32r
    u32 = mybir.dt.uint32

    fhiT = nc.dram_tensor("fhiT", [D, n_shard], f32r,
                          kind="ExternalInput").ap()
    floT = nc.dram_tensor("floT", [D, n_shard], f32r,
                          kind="ExternalInput").ap()
    chiT = nc.dram_tensor("chiT", [D, K], f32r, kind="ExternalInput").ap()
    cloT = nc.dram_tensor("cloT", [D, K], f32r, kind="ExternalInput").ap()
    bhi = nc.dram_tensor("bhi", [1, K], f32r, kind="ExternalInput").ap()
    blo = nc.dram_tensor("blo", [1, K], f32r, kind="ExternalInput").ap()
    ones = nc.dram_tensor("ones", [1, 128], f32r, kind="ExternalInput").ap()
    nh23 = nc.dram_tensor("nh23", [128, K - KH], f32,
                          kind="ExternalInput").ap()
    codes = nc.dram_tensor("codes", [K, D], f32, kind="ExternalInput").ap()
    out = nc.dram_tensor("out", [n_shard, D], f32, kind="ExternalOutput").ap()
    idx_out = nc.dram_tensor("idx_out", [n_shard, 1], u32,
                             kind="ExternalOutput").ap()

    with tile.TileContext(nc) as tc:
        with (
            tc.tile_pool(name="const", bufs=1) as cp,
            tc.tile_pool(name="fin", bufs=3) as fin_pool,
            tc.tile_pool(name="sc", bufs=2) as sc_pool,
            tc.tile_pool(name="rr", bufs=2) as r_pool,
            tc.tile_pool(name="jk", bufs=1) as junk_pool,
            tc.tile_pool(name="small", bufs=3) as small_pool,
            tc.tile_pool(name="gath", bufs=3) as gath_pool,
            tc.tile_pool(name="pdot", bufs=2, space="PSUM") as pdot_pool,
        ):
            chi_sb = cp.tile([D, K], f32r)
            clo_sb = cp.tile([D, K], f32r)
            bhi_sb = cp.tile([1, K], f32r)
            blo_sb = cp.tile([1, K], f32r)
            ones_sb = cp.tile([1, 128], f32r)
            nh23_sb = cp.tile([128, K - KH], f32)
            ninf_sb = cp.tile([128, KH], f32)
            for dst, src in [(chi_sb, chiT), (clo_sb, cloT), (bhi_sb, bhi),
                             (blo_sb, blo), (ones_sb, ones),
                             (nh23_sb, nh23)]:
                nc.sync.dma_start(dst[:], src[:])
            nc.vector.memset(ninf_sb[:], NEG_INF)

            junk = junk_pool.tile([128, K], f32)

            for i in range(m_tiles):
                rows = slice(i * 128, (i + 1) * 128)
                cols = slice(i * 128, (i + 1) * 128)
                fhi_t = fin_pool.tile([D, 128], f32r, tag="fhi")
                flo_t = fin_pool.tile([D, 128], f32r, tag="flo")
                nc.sync.dma_start(fhi_t[:], fhiT[:, cols])
                nc.sync.dma_start(flo_t[:], floT[:, cols])

                pdot = pdot_pool.tile([128, K], f32, tag="dot")
                for c in range(K // KC):
                    sl = slice(c * KC, (c + 1) * KC)
                    fold = c * KC < KH
                    nc.tensor.matmul(pdot[:, sl], fhi_t[:], chi_sb[:, sl],
                                     start=True, stop=False)
                    nc.tensor.matmul(pdot[:, sl], fhi_t[:], clo_sb[:, sl],
                                     start=False, stop=False)
                    nc.tensor.matmul(pdot[:, sl], flo_t[:], chi_sb[:, sl],
                                     start=False, stop=not fold)
                    if fold:
                        nc.tensor.matmul(pdot[:, sl], ones_sb[:],
                                         bhi_sb[:, sl],
                                         start=False, stop=False)
                        nc.tensor.matmul(pdot[:, sl], ones_sb[:],
                                         blo_sb[:, sl],
                                         start=False, stop=True)

                # chunks 2-3: score = dot + (-csq/2) on DVE (PSUM -> SBUF)
                sc23 = sc_pool.tile([128, K - KH], f32, tag="sc23")
                nc.vector.scalar_tensor_tensor(
                    out=sc23[:], in0=pdot[:, KH:K], scalar=1.0,
                    in1=nh23_sb[:], op0=mybir.AluOpType.mult,
                    op1=mybir.AluOpType.add)

                # running max over all K: chunks 0-1 from PSUM, chain 2-3
                r = r_pool.tile([128, K], f32, tag="r")
                nc.vector.tensor_tensor_scan(
                    out=r[:, 0:KH], data0=pdot[:, 0:KH],
                    data1=ninf_sb[:], initial=NEG_INF,
                    op0=mybir.AluOpType.max, op1=mybir.AluOpType.max)
                nc.vector.tensor_tensor_scan(
                    out=r[:, KH:K], data0=sc23[:],
                    data1=ninf_sb[:, 0:K - KH],
                    initial=r[:, KH - 1:KH],
                    op0=mybir.AluOpType.max, op1=mybir.AluOpType.max)

                # ACT: idx = sum_t sign(M - r[t]),  M = r[:, -1]
                idxf = small_pool.tile([128, 1], f32, tag="idxf")
                nc.scalar.activation(
                    out=junk[:], in_=r[:],
                    func=mybir.ActivationFunctionType.Sign,
                    bias=r[:, K - 1:K], scale=-1.0, accum_out=idxf[:])

                idx_u = small_pool.tile([128, 1], u32, tag="idxu")
                nc.vector.tensor_copy(idx_u[:], idxf[:])
                nc.sync.dma_start(idx_out[rows, :], idx_u[:])

                gath = gath_pool.tile([128, D], f32)
                nc.gpsimd.indirect_dma_start(
                    out=gath[:],
                    out_offset=None,
                    in_=codes[:],
                    in_offset=bass.IndirectOffsetOnAxis(ap=idx_u[:, 0:1],
                                                        axis=0),
                )
                nc.sync.dma_start(out[rows, :], gath[:])
    nc.compile()
    return nc


def _get_compiled():
    global _compiled
    if _compiled is None:
        _compiled = _build()
    return _compiled


def kernel(features: np.ndarray, codes: np.ndarray,
           _trace: bool = False, _results_box: list | None = None
           ) -> np.ndarray:
    features = np.ascontiguousarray(features, dtype=np.float32)
    codes = np.ascontiguousarray(codes, dtype=np.float32)
    assert features.shape == (N, D) and codes.shape == (K, D)

    nc = _get_compiled()

    f_hi = _rne11(features)
    f_lo = _rne11(features - f_hi)
    c_hi = _rne11(codes)
    c_lo = _rne11(codes - c_hi)
    csq = (codes.astype(np.float64) ** 2).sum(axis=1)
    nh = (-0.5 * csq).astype(np.float32)
    b_hi = _rne11(nh)
    b_lo = _rne11(nh - b_hi)

    chiT = np.ascontiguousarray(c_hi.T)
    cloT = np.ascontiguousarray(c_lo.T)
    nh23 = np.ascontiguousarray(
        np.broadcast_to(nh[KH:], (128, K - KH)))
    ones = np.ones((1, 128), dtype=np.float32)

    in_maps = []
    for c in range(N_CORES):
        sh = slice(c * N_SHARD, (c + 1) * N_SHARD)
        in_maps.append({
            "fhiT": np.ascontiguousarray(f_hi[sh].T),
            "floT": np.ascontiguousarray(f_lo[sh].T),
            "chiT": chiT,
            "cloT": cloT,
            "bhi": b_hi[None, :],
            "blo": b_lo[None, :],
            "ones": ones,
            "nh23": nh23,
            "codes": codes,
        })
    res = run_bass_kernel_spmd(nc, in_maps, list(range(N_CORES)),
                               trace=_trace)
    if _results_box is not None:
        _results_box.append(res)
    out = np.concatenate([res.results[c]["out"] for c in range(N_CORES)],
                         axis=0)
    return out


if __name__ == "__main__":
    rng = np.random.default_rng(0)
    f = rng.standard_normal((N, D)).astype(np.float32)
    c = rng.standard_normal((K, D)).astype(np.float32)
    got = kernel(f, c)
    d = (f ** 2).sum(1)[:, None] - 2.0 * (f @ c.T) + (c ** 2).sum(1)
    want = c[np.argmin(d, axis=1)]
    err = np.abs(got - want)
    rel = np.linalg.norm(got - want) / np.linalg.norm(want)
    print(f"maxabs={err.max():.3e} rel={rel:.3e} "
          f"badrows={(err.max(1) > 1e-4).sum()}")


# revision 3
# speedup vs baseline: 2.9050x; 1.5660x over previous
"""VQ codebook kernel for Trainium2 (8 NeuronCores, Bass/Tile).

Problem: features [131072, 128] f32, codes [2048, 128] f32.
Output: codes[argmin_k ||f - c_k||^2] -> [131072, 128] f32.

Strategy (data-parallel): shard features N across the 8 cores (16384 rows
each), replicate the codebook. argmin_k dist = argmax_k score, where
score = f.c_k - ||c_k||^2/2.

Host-side prep: features/codes are split hi/lo (fp32r = RNE to 11
mantissa bits; residuals additionally to fp8e5m2), features pre-transposed
per core. Per 128-row tile, entirely on chip:
  - PE (3 matmuls per 512-wide k-chunk, all accumulating into one
    [128,2048] PSUM tile):
      1. f_hi . c_hi          fp32r (1 cyc/row)
      2. f_lo . c_hi + f_hi . c_lo   one fp8e5 DoubleRow matmul
         (0.5 cyc/row; both operands' error products stay ~2^-16)
      3. bias: [1;1] x [b_hi; b_lo]  rank-2 fp32r matmul adds
         -csq/2 at ~2^-22 accuracy in one 1 cyc/row pass
    Total score error ~1e-4 abs -> ~a few argmax flips in 131072 rows.
  - DVE: one running-max tensor_tensor_scan over the PSUM tile -> r,
    row max M = r[:, -1].
  - ACT: idx = sum_t sign(M - r[t]) via one Sign activation with
    accum_out (count of positions before the first max = argmax index,
    matching jnp.argmin first-index tie-break).
  - GPSIMD indirect DMA gathers codes[idx] rows; DMA stores the tile.

Engine busy/tile ~ DVE 2.27us (cap), PE 2.13us, ACT 2.08us, Pool 1.2us.
"""

import os
import sys

import numpy as np

for _p in ("/opt/trn_rl_repo", "/root/.axon_site/_ro/trn_rl_repo"):
    if os.path.isdir(_p) and _p not in sys.path:
        sys.path.insert(0, _p)

import ml_dtypes

import concourse.bacc as bacc
import concourse.bass as bass
import concourse.mybir as mybir
import concourse.tile as tile
from concourse.bass_utils import run_bass_kernel_spmd

N, K, D = 131072, 2048, 128
N_CORES = 8
N_SHARD = N // N_CORES          # 16384
M_TILES = N_SHARD // 128        # 128
KC = 512                        # matmul chunk (one PSUM bank pair)
NEG_INF = -3.0e38
E5 = ml_dtypes.float8_e5m2

_compiled = None


def _rne11(x: np.ndarray) -> np.ndarray:
    """Round fp32 to fp32r: RNE to 11 mantissa bits (drop low 12)."""
    b = np.ascontiguousarray(x, dtype=np.float32).view(np.uint32)
    keep = np.uint32(0xFFFFF000)
    half = np.uint32(0x800)
    tie = (b >> np.uint32(12)) & np.uint32(1)
    r = (b + half - np.uint32(1) + tie) & keep
    return r.view(np.float32)


def _build(n_shard=N_SHARD, num_devices=N_CORES):
    m_tiles = n_shard // 128
    nc = bacc.Bacc("TRN2", target_bir_lowering=False, debug=False,
                   num_devices=num_devices)
    f32 = mybir.dt.float32
    f32r = mybir.dt.float32r
    f8 = mybir.dt.float8e5
    u32 = mybir.dt.uint32

    fhiT = nc.dram_tensor("fhiT", [D, n_shard], f32r,
                          kind="ExternalInput").ap()
    fw8 = nc.dram_tensor("fw8", [D, 2 * n_shard], f8,
                         kind="ExternalInput").ap()
    chiT = nc.dram_tensor("chiT", [D, K], f32r, kind="ExternalInput").ap()
    c8 = nc.dram_tensor("c8", [D, 2 * K], f8, kind="ExternalInput").ap()
    bstack = nc.dram_tensor("bstack", [2, K], f32r,
                            kind="ExternalInput").ap()
    ones2 = nc.dram_tensor("ones2", [2, 128], f32r,
                           kind="ExternalInput").ap()
    codes = nc.dram_tensor("codes", [K, D], f32, kind="ExternalInput").ap()
    out = nc.dram_tensor("out", [n_shard, D], f32, kind="ExternalOutput").ap()

    with tile.TileContext(nc) as tc:
        with (
            tc.tile_pool(name="const", bufs=1) as cp,
            tc.tile_pool(name="fin", bufs=3) as fin_pool,
            tc.tile_pool(name="rr", bufs=2) as r_pool,
            tc.tile_pool(name="jk", bufs=1) as junk_pool,
            tc.tile_pool(name="small", bufs=3) as small_pool,
            tc.tile_pool(name="gath", bufs=3) as gath_pool,
            tc.tile_pool(name="pdot", bufs=2, space="PSUM") as pdot_pool,
        ):
            chi_sb = cp.tile([D, K], f32r)
            c8_sb = cp.tile([D, 2 * K], f8)
            b_sb = cp.tile([2, K], f32r)
            ones2_sb = cp.tile([2, 128], f32r)
            ninf_sb = cp.tile([128, K], f32)
            for dst, src in [(chi_sb, chiT), (c8_sb, c8), (b_sb, bstack),
                             (ones2_sb, ones2)]:
                nc.sync.dma_start(dst[:], src[:])
            nc.vector.memset(ninf_sb[:], NEG_INF)

            junk = junk_pool.tile([128, K], f32)

            for i in range(m_tiles):
                rows = slice(i * 128, (i + 1) * 128)
                fhi_t = fin_pool.tile([D, 128], f32r, tag="fhi")
                fw8_t = fin_pool.tile([D, 256], f8, tag="fw8")
                nc.sync.dma_start(fhi_t[:],
                                  fhiT[:, i * 128:(i + 1) * 128])
                nc.sync.dma_start(fw8_t[:],
                                  fw8[:, i * 256:(i + 1) * 256])
                fw8_3d = fw8_t[:].rearrange("p (two f) -> p two f", two=2)

                pdot = pdot_pool.tile([128, K], f32, tag="dot")
                for c in range(K // KC):
                    sl = slice(c * KC, (c + 1) * KC)
                    nc.tensor.matmul(pdot[:, sl], fhi_t[:], chi_sb[:, sl],
                                     start=True, stop=False)
                    x8 = c8_sb[:, 2 * c * KC:2 * (c + 1) * KC].rearrange(
                        "p (two f) -> p two f", two=2)
                    nc.tensor.matmul(pdot[:, sl], fw8_3d, x8,
                                     start=False, stop=False,
                                     perf_mode=mybir.MatmulPerfMode.DoubleRow)
                    nc.tensor.matmul(pdot[:, sl], ones2_sb[:], b_sb[:, sl],
                                     start=False, stop=True)

                # running max scan over the whole PSUM score tile
                r = r_pool.tile([128, K], f32, tag="r")
                nc.vector.tensor_tensor_scan(
                    out=r[:], data0=pdot[:],
                    data1=ninf_sb[:], initial=NEG_INF,
                    op0=mybir.AluOpType.max, op1=mybir.AluOpType.max)

                # ACT: idx = sum_t sign(M - r[t]),  M = r[:, -1]
                idxf = small_pool.tile([128, 1], f32, tag="idxf")
                nc.scalar.activation(
                    out=junk[:], in_=r[:],
                    func=mybir.ActivationFunctionType.Sign,
                    bias=r[:, K - 1:K], scale=-1.0, accum_out=idxf[:])

                idx_u = small_pool.tile([128, 1], u32, tag="idxu")
                nc.vector.tensor_copy(idx_u[:], idxf[:])

                gath = gath_pool.tile([128, D], f32)
                nc.gpsimd.indirect_dma_start(
                    out=gath[:],
                    out_offset=None,
                    in_=codes[:],
                    in_offset=bass.IndirectOffsetOnAxis(ap=idx_u[:, 0:1],
                                                        axis=0),
                )
                nc.sync.dma_start(out[rows, :], gath[:])
    nc.compile()
    return nc


def _get_compiled():
    global _compiled
    if _compiled is None:
        _compiled = _build()
    return _compiled


def kernel(features: np.ndarray, codes: np.ndarray,
           _trace: bool = False, _results_box: list | None = None
           ) -> np.ndarray:
    features = np.ascontiguousarray(features, dtype=np.float32)
    codes = np.ascontiguousarray(codes, dtype=np.float32)
    assert features.shape == (N, D) and codes.shape == (K, D)

    nc = _get_compiled()

    f_hi = _rne11(features)
    f_lo8 = (features - f_hi).astype(E5)
    f_hi8 = f_hi.astype(E5)
    c_hi = _rne11(codes)
    c_hi8 = c_hi.astype(E5)
    c_lo8 = (codes - c_hi).astype(E5)
    csq = (codes.astype(np.float64) ** 2).sum(axis=1)
    nh = (-0.5 * csq).astype(np.float32)
    b_hi = _rne11(nh)
    b_lo = _rne11(nh - b_hi)

    chiT = np.ascontiguousarray(c_hi.T)
    # c8: per 512-chunk, [c_hi8 | c_lo8] plane pairs: [128, 4, 2, 512]
    c8 = np.empty((D, 2 * K), dtype=E5)
    c8v = c8.reshape(D, K // KC, 2, KC)
    c8v[:, :, 0, :] = c_hi8.T.reshape(D, K // KC, KC)
    c8v[:, :, 1, :] = c_lo8.T.reshape(D, K // KC, KC)
    bstack = np.stack([b_hi, b_lo], axis=0)
    ones2 = np.ones((2, 128), dtype=np.float32)

    in_maps = []
    for c in range(N_CORES):
        sh = slice(c * N_SHARD, (c + 1) * N_SHARD)
        fhiT = np.ascontiguousarray(f_hi[sh].T)
        # fw8: per 128-row tile, [f_lo8.T | f_hi8.T] plane pairs
        fw8 = np.empty((D, 2 * N_SHARD), dtype=E5)
        fv = fw8.reshape(D, M_TILES, 2, 128)
        fv[:, :, 0, :] = f_lo8[sh].T.reshape(D, M_TILES, 128)
        fv[:, :, 1, :] = f_hi8[sh].T.reshape(D, M_TILES, 128)
        in_maps.append({
            "fhiT": fhiT,
            "fw8": fw8,
            "chiT": chiT,
            "c8": c8,
            "bstack": bstack,
            "ones2": ones2,
            "codes": codes,
        })
    res = run_bass_kernel_spmd(nc, in_maps, list(range(N_CORES)),
                               trace=_trace)
    if _results_box is not None:
        _results_box.append(res)
    out = np.concatenate([res.results[c]["out"] for c in range(N_CORES)],
                         axis=0)
    return out


if __name__ == "__main__":
    rng = np.random.default_rng(0)
    f = rng.standard_normal((N, D)).astype(np.float32)
    c = rng.standard_normal((K, D)).astype(np.float32)
    got = kernel(f, c)
    d = (f ** 2).sum(1)[:, None] - 2.0 * (f @ c.T) + (c ** 2).sum(1)
    want = c[np.argmin(d, axis=1)]
    err = np.abs(got - want)
    rel = np.linalg.norm(got - want) / np.linalg.norm(want)
    print(f"maxabs={err.max():.3e} rel={rel:.3e} "
          f"badrows={(err.max(1) > 1e-4).sum()}")
